# revision 72
# baseline (speedup 1.0000x reference)
"""Causal self-attention (B=1, T=4096, C=1024, H=8) on 8 trn2 NeuronCores.

Tensor-parallel over heads: core h owns head h (D=128 = partition width).
Everything is computed feature-major ("transposed") so the PE contraction
dim always sits on SBUF partitions.

v22: fp8 matmuls for the bulk of the work (DoubleRow for QKV/AV/
denominator = 2 contraction tiles per pass; plain fp8 for scores),
plus a schedule that keeps the PE stream dense and the ACT/DVE/GpSimd
engines balanced (~85/91/19us in a ~127us PE window):
  - Only query chunk 0 (tokens < 512) stays bf16 end-to-end: max-error
    is dominated by early tokens whose softmax support is too small to
    average fp8 quantization noise. Chunk 1 and later use fp8 x / fp8
    weights (x16) for QKV, fp8 q/k for scores (kT8 copy; pair 0 reads
    a bf16 kTb copy), and fp8 exp(att) + fp8 v (x16) for both the AV
    and the softmax-denominator matmuls.
  - Scores A/B halves live in independent single-bank PSUM tiles
    (tags s2A/s2B, 2 bufs each) with per-half exp instructions, so
    each half's scores(si) only waits on its own exp(si-2) instead of
    the combined [A|B] exp.
  - Exp is split across engines: A-half exps on ACT (true exp), B-half
    exps of fp8 pairs on DVE as Schraudolph tensor_scalar
    (s + SCH_BIAS/S1) * S1 -> uint8 -> bitcast e4m3. The f32->u8
    conversion saturates (negatives -> 0, round-to-nearest), so no max
    op is needed.
  - NO mask matmuls at all: diag tile j's fully-masked first 128j
    columns are skipped by the score matmul and exp (GpSimd memsets
    the p2 prefix to 0), and the remaining 128-wide triangle is zeroed
    POST-exp by a GpSimd affine_select on the p2 tile. exp of unmasked
    scores cannot overflow (bf16 range; fp8/u8 saturate).
  - AV + denominator are emitted two iterations late (software
    pipelining) so the in-order PE queue never parks on an exp wait.
  - c_proj of pair pc is deferred into pair pc+1's inner loop; each
    unit's two PSUM->SBUF copies are split across DVE and ACT so they
    drain concurrently and release the shared s2A/s2B PSUM slots (and
    with them the scores pipeline) sooner. QKV unit epilogues ride ACT
    via the activation bias port (Identity + per-partition bias AP).
  - Activation DMAs use 1-2 descriptors per tensor (not per 128-row
    block): the Sync engine's ~650ns/descriptor issue rate was pacing
    the startup QKV.

Scale bookkeeping: wq/wk/wv are pre-scaled x16 (bf16 and fp8 copies), so
logits are 256x -- folded into the exp scale (free on ACT) and into
SCH_S1 on the DVE path. v is stored 16x; the denominator stationary
holds 16.0, so yT = (16 p@v)/(16 sum p) comes out natural. The k-bias
is dropped (softmax shift-invariance).

Per core the output partial (c_proj columns of this head only) is
written as bf16; host sums the 8 partials in f32, adds b_proj.
"""

import math
import os
import sys

for _p in ("/opt/trn_rl_repo",):
    if _p not in sys.path:
        sys.path.insert(0, _p)

import numpy as np
import ml_dtypes

import concourse.bass as bass
import concourse.mybir as mybir
import concourse.tile as tile
from concourse import bacc
from concourse import bass_utils
from concourse.masks import make_identity

B, T, C, H = 1, 4096, 1024, 8
D = C // H          # 128, head dim == partition width
N_CORES = 8
TQ = 512            # query-chunk (matmul moving free dim)
CO = C // 128      # 8 contraction tiles of 128
F32 = mybir.dt.float32
BF16 = mybir.dt.bfloat16
F8 = mybir.dt.float8e4
DR = mybir.MatmulPerfMode.DoubleRow

SW = 16.0           # weight / v scale for fp8 range
NEG = -1.0e9        # additive causal mask value

# knobs
FP8_QKV = True      # fp8 DoubleRow QKV for chunk pairs >= 1
FP8_AV = True       # fp8 DoubleRow AV + denominator for chunk pairs >= 1


def _np_dt(dt):
    return {F32: np.float32, BF16: ml_dtypes.bfloat16,
            F8: ml_dtypes.float8_e4m3}[dt]


def build(t_len=T):
    """Emit the single-core SPMD program (same code on all 8 cores)."""
    n_chunks = t_len // TQ
    n_pairs = n_chunks // 2   # query chunks processed in pairs of 2*TQ cols
    n_ttiles = t_len // 128
    T2 = 2 * TQ
    exp_scale = (1.0 / math.sqrt(D)) / (SW * SW)
    # Schraudolph exp-to-fp8 on DVE: I = max(s2*SCH_S1, 0) as uint8, where
    # the mask matmul pre-added SCH_BIAS/SCH_S1; bitcast uint8 -> e4m3.
    global SCH_S1, SCH_BIAS
    SCH_S1 = exp_scale * 8.0 / math.log(2.0)
    SCH_BIAS = 56.0 + 1.2   # +0.5 for truncating float->int conversion

    nc = bacc.Bacc(
        "TRN2", target_bir_lowering=False, debug=False, num_devices=N_CORES
    )

    # chunk-0 (tokens < TQ) inputs, bf16: early queries have softmax support
    # too small to average fp8 quantization noise, so chunk 0 stays bf16
    # end-to-end. Chunk 1 (tokens TQ..T2) and later are fp8.
    # all activation/weight inputs are host-prearranged to the SBUF
    # [partition, o, t] layout so the DMAs read contiguous 8KB runs per
    # partition (the (o p) t -> p o t rearrange on the DRAM side decomposed
    # into 256B packets at ~16GB/s/engine and paced the startup)
    x0A_d = nc.dram_tensor("x0A", [128, CO, TQ], BF16, kind="ExternalInput")
    wqb_d = nc.dram_tensor("wqb", [128, CO, D], BF16, kind="ExternalInput")
    wkb_d = nc.dram_tensor("wkb", [128, CO, D], BF16, kind="ExternalInput")
    wvb_d = nc.dram_tensor("wvb", [128, CO, D], BF16, kind="ExternalInput")
    # chunk-1 input (fp8); x8T holds tokens T2..t_len, pair-major
    x08_d = nc.dram_tensor("x08", [128, CO, TQ], F8, kind="ExternalInput")
    wq8_d = nc.dram_tensor("wq8", [128, CO, D], F8, kind="ExternalInput")
    wk8_d = nc.dram_tensor("wk8", [128, CO, D], F8, kind="ExternalInput")
    wv8_d = nc.dram_tensor("wv8", [128, CO, D], F8, kind="ExternalInput")
    if n_pairs > 1:
        x8T_d = nc.dram_tensor("x8T", [128, (n_pairs - 1) * CO, T2], F8,
                               kind="ExternalInput")
    wp_d = nc.dram_tensor("wp", [D, C], BF16, kind="ExternalInput")
    bq_d = nc.dram_tensor("bq", [D, 1], F32, kind="ExternalInput")
    bv_d = nc.dram_tensor("bv", [D, 1], F32, kind="ExternalInput")
    outP_d = nc.dram_tensor("outP", [C, t_len], BF16, kind="ExternalOutput")

    with tile.TileContext(nc) as tc:
        with (
            tc.tile_pool(name="const", bufs=1) as cpool,
            tc.tile_pool(name="persist", bufs=1) as ppool,
            tc.tile_pool(name="work", bufs=2) as wpool,
            tc.tile_pool(name="ptiles", bufs=3) as pt_pool,
            tc.tile_pool(name="psum", bufs=1, space="PSUM") as psum,
        ):
            # ---- constants / weights -------------------------------------
            # wqb first so the very first (bf16 chunk-0 QKV) matmuls are
            # unblocked asap
            wqb_sb = cpool.tile([128, CO, D], BF16, name="wqb_sb")
            wkb_sb = cpool.tile([128, CO, D], BF16, name="wkb_sb")
            wvb_sb = cpool.tile([128, CO, D], BF16, name="wvb_sb")
            wq8_sb = cpool.tile([128, CO, D], F8, name="wq8_sb")
            wk8_sb = cpool.tile([128, CO, D], F8, name="wk8_sb")
            wv8_sb = cpool.tile([128, CO, D], F8, name="wv8_sb")
            wp_sb = cpool.tile([128, CO, D], BF16, name="wp_sb")
            nc.sync.dma_start(wqb_sb[:], wqb_d.ap())
            bq_sb = cpool.tile([D, 1], F32, name="bq_sb")
            bv_sb = cpool.tile([D, 1], F32, name="bv_sb")
            nc.sync.dma_start(bq_sb[:], bq_d.ap())
            nc.sync.dma_start(bv_sb[:], bv_d.ap())

            # denominator stationary tiles hold 16.0 so sums = 16*sum(p);
            # memset first so the PE warmup below is unblocked immediately
            sixt8 = cpool.tile([128, 2, 128], F8, name="sixt8")
            nc.vector.memset(sixt8[:], SW)
            sixtb = cpool.tile([128, 128], BF16, name="sixtb")
            nc.vector.memset(sixtb[:], SW)
            ident = cpool.tile([128, 128], BF16, name="ident")
            make_identity(nc, ident[:])
            # HAM/ifetch warmup: dummy DR matmuls while input DMAs land
            for wi in range(24):
                warm_ps = psum.tile([128, 128], F32,
                                    tag="s2A" if wi % 2 == 0 else "s2B",
                                    name="warm_ps", bufs=2)
                nc.tensor.matmul(warm_ps[:], sixt8[:], sixt8[:],
                                 start=True, stop=True, perf_mode=DR)

            # ---- persistent activations ----------------------------------
            kT8_sb = ppool.tile([128, t_len], F8, name="kT8_sb")
            kTb_sb = ppool.tile([128, T2], BF16, name="kTb_sb")
            v8_sb = ppool.tile([128, n_ttiles, D], F8, name="v8_sb")
            vb_sb = ppool.tile([128, 4, D], BF16, name="vb_sb")
            yT_sb = ppool.tile([128, t_len], BF16, name="yT_sb")


            x0A = wpool.tile([128, CO, TQ], BF16, tag="x0A", name="x0A", bufs=1)
            nc.sync.dma_start(x0A[:, 0:4, :], x0A_d.ap()[:, 0:4, :])
            nc.sync.dma_start(x0A[:, 4:CO, :], x0A_d.ap()[:, 4:CO, :])
            for w_sb, w_d in ((wkb_sb, wkb_d), (wvb_sb, wvb_d)):
                nc.sync.dma_start(w_sb[:], w_d.ap())
            xc0 = wpool.tile([128, CO, TQ], F8, tag="xc0", name="xc0", bufs=1)
            nc.sync.dma_start(xc0[:, 0:4, :], x08_d.ap()[:, 0:4, :])
            nc.sync.dma_start(xc0[:, 4:CO, :], x08_d.ap()[:, 4:CO, :])
            for w_sb, w_d in ((wq8_sb, wq8_d), (wk8_sb, wk8_d),
                              (wv8_sb, wv8_d)):
                nc.sync.dma_start(w_sb[:], w_d.ap())
            nc.sync.dma_start(
                wp_sb[:], wp_d.ap().rearrange("d (o j) -> d o j", j=128)
            )

            def make_proj_units(pj, half):
                # c_proj of pair pj, one chunk-half, as 4 deferred units.
                # Each unit computes TWO output column tiles into one 2-bank
                # PSUM tile (single pool allocation), one PSUM->SBUF copy
                # (bf16), one DMA. Units are drained one per inner-loop
                # iteration so the copies and DMAs spread out.
                units = []
                lo = pj * T2 + half * TQ
                for j0 in range(0, CO, 2):
                    def unit(lo=lo, j0=j0, half=half):
                        outc = wpool.tile([128, 2, TQ], BF16, tag="outc",
                                          name="outc", bufs=4)
                        for jj, ptag in ((0, "s2A"), (1, "s2B")):
                            oh = psum.tile([128, TQ], F32, tag=ptag,
                                           name="oh", bufs=2)
                            nc.tensor.matmul(
                                oh[:], wp_sb[:, j0 + jj, :],
                                yT_sb[:, lo : lo + TQ],
                                start=True, stop=True,
                            )
                            # split the unit's two copies across ACT and
                            # DVE so they run concurrently and free the
                            # scores' s2A/s2B PSUM slots sooner
                            if jj == 0:
                                nc.vector.tensor_copy(outc[:, jj, :], oh[:])
                            else:
                                nc.scalar.copy(outc[:, jj, :], oh[:])
                        nc.sync.dma_start(
                            outP_d.ap()[j0 * 128 : (j0 + 2) * 128,
                                        lo : lo + TQ]
                            .rearrange("(o p) t -> p o t", p=128),
                            outc[:],
                        )
                    units.append(unit)
                return units

            pair_state = {}

            def make_qkv_units(pj, xc, halves=(0, 1), xoff=0):
                # fp8 DoubleRow QKV for pair pj as single-bank deferred
                # units (kind x half); drained inside pair pj-1's loop.
                # xoff: token offset of xc's first column within the pair.
                t0p = pj * T2
                qT = wpool.tile([128, T2], BF16 if pj == 0 else F8,
                                tag="qT", name="qT", bufs=2)
                vT = wpool.tile([128, T2], BF16, tag="vT", name="vT", bufs=2)
                pair_state[pj] = (qT, vT)
                units = []
                # q first (next pair's scores need it at sp=0), then v
                # (transposes at sp=1), then k (scores si >= n_sA at sp~4)
                for kind in (0, 2, 1):   # 0=q, 1=k, 2=v
                    for half in halves:
                        def unit(kind=kind, half=half):
                            hs = slice(half * TQ, (half + 1) * TQ)
                            xs = slice(half * TQ - xoff,
                                       (half + 1) * TQ - xoff)
                            dst = psum.tile([128, TQ], F32,
                                            tag="s2A" if half == 0 else "s2B",
                                            name="qkvu", bufs=2)
                            w_sb = (wq8_sb, wk8_sb, wv8_sb)[kind]
                            for op in range(CO // 2):
                                o = 2 * op
                                nc.tensor.matmul(
                                    dst[:], w_sb[:, o : o + 2, :],
                                    xc[:, o : o + 2, xs],
                                    start=(op == 0), stop=(op == CO // 2 - 1),
                                    perf_mode=DR,
                                )
                            # unit epilogues ride ACT (bias via the
                            # activation bias port): the DVE queue stays
                            # short for the schrau exps, and the unit's
                            # s2A/s2B PSUM slot frees sooner
                            if kind == 0:
                                nc.scalar.activation(
                                    qT[:, hs], dst[:],
                                    mybir.ActivationFunctionType.Copy,
                                    bias=bq_sb[:, 0:1],
                                )
                            elif kind == 1:
                                nc.scalar.copy(
                                    kT8_sb[:, t0p + half * TQ
                                           : t0p + (half + 1) * TQ],
                                    dst[:],
                                )
                                if pj == 0:
                                    nc.vector.tensor_copy(
                                        kTb_sb[:, t0p + half * TQ
                                               : t0p + (half + 1) * TQ],
                                        dst[:],
                                    )
                            else:
                                nc.scalar.activation(
                                    vT[:, hs], dst[:],
                                    mybir.ActivationFunctionType.Copy,
                                    bias=bv_sb[:, 0:1],
                                )
                        units.append(unit)
                return units

            pending = []

            def drain_one():
                if pending:
                    pending.pop(0)()

            # pair 0's QKV inline: chunk 1's fp8 DR units first (xc0 is the
            # smaller, earliest-landing DMA), then chunk 0 in bf16
            qkv0_units = make_qkv_units(0, xc0, halves=(1,), xoff=TQ)
            qT0, vT0 = pair_state[0]
            A0 = slice(0, TQ)
            # fp8 B units first: xc0 (256KB fp8) lands well before the
            # 1MB bf16 x0A, so they fill the x0A DMA wait
            for u in qkv0_units:
                u()
            for kind, w_sb in ((0, wqb_sb), (1, wkb_sb), (2, wvb_sb)):
                dst0 = psum.tile([128, TQ], F32,
                                 tag="s2A" if kind != 1 else "s2B",
                                 name="dst0", bufs=2)
                for o in range(CO):
                    nc.tensor.matmul(
                        dst0[:], w_sb[:, o, :], x0A[:, o, :],
                        start=(o == 0), stop=(o == CO - 1),
                    )
                if kind == 0:
                    nc.vector.tensor_add(
                        qT0[:, A0], dst0[:],
                        bq_sb[:, 0:1].to_broadcast([D, TQ])
                    )
                elif kind == 1:
                    nc.vector.tensor_copy(kTb_sb[:, 0:TQ], dst0[:])
                    nc.vector.tensor_copy(kT8_sb[:, 0:TQ], dst0[:])
                else:
                    nc.vector.tensor_add(
                        vT0[:, A0], dst0[:],
                        bv_sb[:, 0:1].to_broadcast([D, TQ])
                    )

            for pc in range(n_pairs):
                t0 = pc * T2           # start of chunk A; chunk B at t0+TQ
                fp8av = FP8_AV
                # next pair's x chunk DMA + its deferred QKV units
                if pc + 1 < n_pairs:
                    t0n = (pc + 1) * T2
                    xc_next = wpool.tile([128, CO, T2], F8, tag="xc",
                                         name="xc", bufs=2)
                    nc.sync.dma_start(
                        xc_next[:], x8T_d.ap()[:, pc * CO : (pc + 1) * CO, :]
                    )
                    qkv_units_next = make_qkv_units(pc + 1, xc_next)
                else:
                    qkv_units_next = []

                qT_cur, vT_tmp = pair_state[pc]

                def emit_transposes():
                    for vg in range(2):
                        vt_ps = psum.tile([128, 4, 128], BF16,
                                          tag="s2A" if vg == 0 else "s2B",
                                          name="vt_ps", bufs=2)
                        for tt in range(4):
                            col = (vg * 4 + tt) * 128
                            nc.tensor.transpose(
                                vt_ps[:, tt, :], vT_tmp[:, col : col + 128],
                                ident[:],
                            )
                        base = pc * 8 + vg * 4
                        nc.vector.tensor_copy(
                            v8_sb[:, base : base + 4, :], vt_ps[:]
                        )
                        if pc == 0 and vg == 0:
                            # bf16 v for chunk 0's own (bf16) AV
                            nc.vector.tensor_copy(vb_sb[:], vt_ps[:])

                # ---- attention for the pair ------------------------------
                n_sA = (t0 + TQ) // 128        # s-tiles for chunk A
                n_sB = (t0 + T2) // 128        # s-tiles for chunk B
                yAB = psum.tile([128, T2], F32, tag="yAB", name="yAB", bufs=1)
                sumAB = psum.tile([128, T2], F32, tag="sumAB", name="sumAB",
                                  bufs=1)
                A, Bh = slice(0, TQ), slice(TQ, T2)
                recip = wpool.tile([128, T2], F32, tag="recip", name="recip",
                                   bufs=2)
                if pc == 0:
                    emit_transposes()   # pair 0's AV needs own v from si=0

                n_sp = n_sB // 2
                hyb = pc == 0   # chunk 0 (A half of pair 0) stays bf16

                def make_av(sp, p2, p2b=None):
                    # AV + denominator matmuls for step sp; emitted one
                    # iteration late (software pipelining) so the in-order PE
                    # queue never parks on an exp wait while later-emitted
                    # ready work exists.
                    si0 = 2 * sp
                    in_A = si0 < n_sA

                    def emit():
                        for hsl, n_s, last in (
                            (A, n_sA, in_A and sp == n_sA // 2 - 1),
                            (Bh, n_sB, sp == n_sp - 1),
                        ):
                            if hsl is A and not in_A:
                                continue
                            if hsl is A and hyb:
                                # chunk 0: bf16 p x bf16 v, no DoubleRow
                                for sl_i in range(2):
                                    si = si0 + sl_i
                                    nc.tensor.matmul(
                                        sumAB[:, A], sixtb[:], p2b[:, sl_i, :],
                                        start=(si == 0), stop=(si == n_sA - 1),
                                    )
                                    nc.tensor.matmul(
                                        yAB[:, A], vb_sb[:, si, :],
                                        p2b[:, sl_i, :],
                                        start=(si == 0), stop=(si == n_sA - 1),
                                    )
                                continue
                            nc.tensor.matmul(
                                sumAB[:, hsl], sixt8[:], p2[:, :, hsl],
                                start=(sp == 0), stop=last, perf_mode=DR,
                            )
                            nc.tensor.matmul(
                                yAB[:, hsl], v8_sb[:, si0 : si0 + 2, :],
                                p2[:, :, hsl],
                                start=(sp == 0), stop=last, perf_mode=DR,
                            )
                        if in_A and si0 == n_sA - 2:
                            # A-half AV complete: normalize early, then queue
                            # the A-half c_proj to fill later iterations
                            nc.vector.reciprocal_approx_fast(recip[:, A],
                                                             sumAB[:, A])
                            nc.vector.tensor_mul(
                                yT_sb[:, t0 : t0 + TQ], yAB[:, A], recip[:, A]
                            )
                            pending.extend(make_proj_units(pc, 0))

                    return emit

                av_prev = None
                av_prev2 = None
                for sp in range(n_sp):
                    if sp == max(0, n_sp - 7):
                        pending.extend(qkv_units_next)
                        qkv_units_next = []
                    si0 = 2 * sp
                    in_A = si0 < n_sA    # n_sA is a multiple of 4
                    p2 = pt_pool.tile([128, 2, T2], F8, tag="p28", name="p2",
                                      bufs=4)
                    p2b = None
                    if hyb and in_A:
                        p2b = pt_pool.tile([128, 2, TQ], BF16, tag="p2b",
                                           name="p2b", bufs=3)
                    for sl_i in range(2):
                        si = si0 + sl_i
                        s0 = si * 128
                        diagA = in_A and si >= n_sA - 4
                        diagB = si >= n_sB - 4
                        # diag tile j: cols < 128j are fully masked -- skip
                        # them in the score/exp stream (p2 prefix zeroed by
                        # the idle GpSimd) and mask only the 128-wide
                        # triangle
                        cA = 128 * (si - (n_sA - 4)) if diagA else 0
                        cB = 128 * (si - (n_sB - 4)) if diagB else 0
                        # scores A and B live in independent single-bank
                        # PSUM tiles with per-half exps, so each half's
                        # scores(si) only waits on its own exp(si-2)
                        def tri_zero(ap):
                            # zero p2's diagonal 128x128 triangle (q < k)
                            # post-exp on the idle GpSimd: replaces the PE
                            # mask matmul entirely. exp of unmasked scores
                            # can't overflow (bf16 range; fp8/u8 saturate).
                            nc.gpsimd.affine_select(
                                out=ap, in_=ap,
                                compare_op=mybir.AluOpType.is_ge, fill=0.0,
                                base=0, pattern=[[1, 128]],
                                channel_multiplier=-1,
                            )

                        if in_A:
                            sA = psum.tile([128, TQ], F32, tag="s2A",
                                           name="sA", bufs=2)
                            if cA:
                                nc.gpsimd.memset(
                                    (p2b if hyb else p2)[:, sl_i, 0:cA], 0.0
                                )
                            nc.tensor.matmul(sA[:, cA:],
                                             (kTb_sb if hyb else kT8_sb)
                                             [:, s0 : s0 + 128],
                                             qT_cur[:, cA:TQ], start=True,
                                             stop=True)
                            nc.scalar.activation(
                                (p2b[:, sl_i, cA:] if hyb
                                 else p2[:, sl_i, cA:TQ]),
                                sA[:, cA:],
                                mybir.ActivationFunctionType.Exp,
                                scale=exp_scale,
                            )
                            if diagA:
                                tri_zero((p2b if hyb else p2)
                                         [:, sl_i, cA : cA + 128])
                        sB = psum.tile([128, TQ], F32, tag="s2B",
                                       name="sB", bufs=2)
                        if cB:
                            nc.gpsimd.memset(p2[:, sl_i, TQ : TQ + cB], 0.0)
                        nc.tensor.matmul(sB[:, cB:],
                                         (kTb_sb if hyb else kT8_sb)
                                         [:, s0 : s0 + 128],
                                         qT_cur[:, TQ + cB : T2], start=True,
                                         stop=True)
                        if hyb:
                            # pair 0's B half (chunk 1) keeps the accurate
                            # ACT exp: it is the accuracy-thinnest fp8 chunk
                            nc.scalar.activation(
                                p2[:, sl_i, TQ + cB : T2], sB[:, cB:],
                                mybir.ActivationFunctionType.Exp,
                                scale=exp_scale,
                            )
                        else:
                            # fp8 pairs: Schraudolph exp on DVE. f32->u8
                            # conversion saturates ([0,255], negatives -> 0),
                            # so (s + B)*S1 needs no max op. Keeps the
                            # Scalar engine free for the A-half exps.
                            nc.vector.tensor_scalar(
                                p2[:, sl_i, TQ + cB : T2]
                                .bitcast(mybir.dt.uint8),
                                sB[:, cB:], SCH_BIAS / SCH_S1, SCH_S1,
                                op0=mybir.AluOpType.add,
                                op1=mybir.AluOpType.mult,
                            )
                        if diagB:
                            tri_zero(p2[:, sl_i, TQ + cB : TQ + cB + 128])
                    if av_prev2 is not None:
                        av_prev2()
                    av_prev2 = av_prev
                    av_prev = make_av(sp, p2, p2b)
                    drain_one()
                    if pc == 0 or (pc == n_pairs - 1 and sp >= n_sp - 4):
                        drain_one()   # ramp/tail regions are PE-light
                    if pc > 0 and sp == 1:
                        # own-pair v only needed from si >= n_sA; transposing
                        # here hides the vT copyback latency behind scores
                        emit_transposes()

                if av_prev2 is not None:
                    av_prev2()
                av_prev()
                # normalize B first so its DVE ops aren't queued behind the
                # drained units' copies, then flush the queue (QKV of the
                # next pair, proj leftovers)
                nc.vector.reciprocal_approx_fast(recip[:, Bh], sumAB[:, Bh])
                nc.vector.tensor_mul(
                    yT_sb[:, t0 + TQ : t0 + T2], yAB[:, Bh], recip[:, Bh]
                )
                while pending:
                    drain_one()
                pending.extend(make_proj_units(pc, 1))

            # last pair's remaining c_proj
            while pending:
                drain_one()

    nc.compile()
    return nc


def make_in_maps(x, w_attn, b_attn, w_proj, b_proj, t_len=T):
    """Shard + lay out the full inputs for the 8 cores."""
    x = np.asarray(x, dtype=np.float32).reshape(t_len, C)
    w_attn = np.asarray(w_attn, dtype=np.float32)
    b_attn = np.asarray(b_attn, dtype=np.float32)
    w_proj = np.asarray(w_proj, dtype=np.float32)

    bf = ml_dtypes.bfloat16
    f8 = ml_dtypes.float8_e4m3
    xT = np.ascontiguousarray(x.T)

    def _po(arr):
        # [C, X] -> SBUF layout [128, CO, X] (partition-major)
        return np.ascontiguousarray(
            arr.reshape(CO, 128, arr.shape[1]).transpose(1, 0, 2))

    T2 = 2 * TQ
    x0A = _po(xT[:, :TQ]).astype(bf)
    x08 = _po(xT[:, TQ:T2]).astype(f8)
    has8 = t_len > T2
    if has8:
        n_p1 = (t_len - T2) // T2
        x8T = np.concatenate(
            [_po(xT[:, T2 * (i + 1) : T2 * (i + 2)]) for i in range(n_p1)],
            axis=1).astype(f8)

    in_maps = []
    for h in range(N_CORES):
        sl = slice(h * D, (h + 1) * D)
        wq = np.ascontiguousarray((w_attn[sl, :] * SW).T)
        wk = np.ascontiguousarray((w_attn[C + h * D : C + (h + 1) * D, :] * SW).T)
        wv = np.ascontiguousarray((w_attn[2 * C + h * D : 2 * C + (h + 1) * D, :] * SW).T)
        wp = np.ascontiguousarray(w_proj[:, sl].T).astype(bf)
        m = {
            "x0A": x0A,
            "x08": x08,
            "wqb": _po(wq).astype(bf), "wkb": _po(wk).astype(bf),
            "wvb": _po(wv).astype(bf),
            "wp": wp,
            "bq": (b_attn[sl] * SW).reshape(D, 1).astype(np.float32),
            "bv": (b_attn[2 * C + h * D : 2 * C + (h + 1) * D] * SW)
                  .reshape(D, 1).astype(np.float32),
            "wq8": _po(wq).astype(f8),
            "wk8": _po(wk).astype(f8),
            "wv8": _po(wv).astype(f8),
        }
        if has8:
            m["x8T"] = x8T
        in_maps.append(m)
    return in_maps


_COMPILED = {}


def _get_compiled(t_len=T):
    if t_len not in _COMPILED:
        _COMPILED[t_len] = build(t_len)
    return _COMPILED[t_len]


def kernel(x, w_attn, b_attn, w_proj, b_proj, trace=False):
    nc = _get_compiled()
    in_maps = make_in_maps(x, w_attn, b_attn, w_proj, b_proj)
    res = bass_utils.run_bass_kernel_spmd(
        nc, in_maps, core_ids=list(range(N_CORES)), trace=trace
    )
    acc = res.results[0]["outP"].astype(np.float32)
    for h in range(1, N_CORES):
        acc += res.results[h]["outP"].astype(np.float32)
    out = acc.T + np.asarray(b_proj, dtype=np.float32)
    out = np.ascontiguousarray(out, dtype=np.float32).reshape(B, T, C)
    if trace:
        kernel.last_exec_time_ns = res.exec_time_ns
        kernel.last_results = res
    return out



# revision 73
# speedup vs baseline: 1.1847x; 1.1847x over previous
"""Causal self-attention (B=1, T=4096, C=1024, H=8) on 8 trn2 NeuronCores.

Tensor-parallel over heads: core h owns head h (D=128 = partition width).
Everything is computed feature-major ("transposed") so the PE contraction
dim always sits on SBUF partitions.

v22: fp8 matmuls for the bulk of the work (DoubleRow for QKV/AV/
denominator = 2 contraction tiles per pass; plain fp8 for scores),
plus a schedule that keeps the PE stream dense and the ACT/DVE/GpSimd
engines balanced (~85/91/19us in a ~127us PE window):
  - Only query chunk 0 (tokens < 512) stays bf16 end-to-end: max-error
    is dominated by early tokens whose softmax support is too small to
    average fp8 quantization noise. Chunk 1 and later use fp8 x / fp8
    weights (x16) for QKV, fp8 q/k for scores (kT8 copy; pair 0 reads
    a bf16 kTb copy), and fp8 exp(att) + fp8 v (x16) for both the AV
    and the softmax-denominator matmuls.
  - Scores A/B halves live in independent single-bank PSUM tiles
    (tags s2A/s2B, 2 bufs each) with per-half exp instructions, so
    each half's scores(si) only waits on its own exp(si-2) instead of
    the combined [A|B] exp.
  - Exp is split across engines: A-half exps on ACT (true exp), B-half
    exps of fp8 pairs on DVE as Schraudolph tensor_scalar
    (s + SCH_BIAS/S1) * S1 -> uint8 -> bitcast e4m3. The f32->u8
    conversion saturates (negatives -> 0, round-to-nearest), so no max
    op is needed.
  - NO mask matmuls at all: diag tile j's fully-masked first 128j
    columns are skipped by the score matmul and exp (GpSimd memsets
    the p2 prefix to 0), and the remaining 128-wide triangle is zeroed
    POST-exp by a GpSimd affine_select on the p2 tile. exp of unmasked
    scores cannot overflow (bf16 range; fp8/u8 saturate).
  - AV + denominator are emitted two iterations late (software
    pipelining) so the in-order PE queue never parks on an exp wait.
  - c_proj of pair pc is deferred into pair pc+1's inner loop; each
    unit's two PSUM->SBUF copies are split across DVE and ACT so they
    drain concurrently and release the shared s2A/s2B PSUM slots (and
    with them the scores pipeline) sooner. QKV unit epilogues ride ACT
    via the activation bias port (Identity + per-partition bias AP).
  - Activation DMAs use 1-2 descriptors per tensor (not per 128-row
    block): the Sync engine's ~650ns/descriptor issue rate was pacing
    the startup QKV.

Scale bookkeeping: wq/wk/wv are pre-scaled x16 (bf16 and fp8 copies), so
logits are 256x -- folded into the exp scale (free on ACT) and into
SCH_S1 on the DVE path. v is stored 16x; the denominator stationary
holds 16.0, so yT = (16 p@v)/(16 sum p) comes out natural. The k-bias
is dropped (softmax shift-invariance).

Per core the output partial (c_proj columns of this head only) is
written as bf16; host sums the 8 partials in f32, adds b_proj.
"""

import math
import os
import sys

for _p in ("/opt/trn_rl_repo",):
    if _p not in sys.path:
        sys.path.insert(0, _p)

import numpy as np
import ml_dtypes

import concourse.bass as bass
import concourse.mybir as mybir
import concourse.tile as tile
from concourse import bacc
from concourse import bass_utils
from concourse.masks import make_identity

B, T, C, H = 1, 4096, 1024, 8
D = C // H          # 128, head dim == partition width
N_CORES = 8
TQ = 512            # query-chunk (matmul moving free dim)
CO = C // 128      # 8 contraction tiles of 128
F32 = mybir.dt.float32
BF16 = mybir.dt.bfloat16
F8 = mybir.dt.float8e4
DR = mybir.MatmulPerfMode.DoubleRow

SW = 16.0           # weight / v scale for fp8 range
NEG = -1.0e9        # additive causal mask value

# knobs
FP8_QKV = True      # fp8 DoubleRow QKV for chunk pairs >= 1
FP8_AV = True       # fp8 DoubleRow AV + denominator for chunk pairs >= 1


def _np_dt(dt):
    return {F32: np.float32, BF16: ml_dtypes.bfloat16,
            F8: ml_dtypes.float8_e4m3}[dt]


def build(t_len=T):
    """Emit the single-core SPMD program (same code on all 8 cores)."""
    n_chunks = t_len // TQ
    n_pairs = n_chunks // 2   # query chunks processed in pairs of 2*TQ cols
    n_ttiles = t_len // 128
    T2 = 2 * TQ
    exp_scale = (1.0 / math.sqrt(D)) / (SW * SW)
    # Schraudolph exp-to-fp8 on DVE: I = max(s2*SCH_S1, 0) as uint8, where
    # the mask matmul pre-added SCH_BIAS/SCH_S1; bitcast uint8 -> e4m3.
    global SCH_S1, SCH_BIAS
    SCH_S1 = exp_scale * 8.0 / math.log(2.0)
    SCH_BIAS = 56.0 + 1.2   # +0.5 for truncating float->int conversion

    nc = bacc.Bacc(
        "TRN2", target_bir_lowering=False, debug=False, num_devices=N_CORES
    )

    # chunk-0 (tokens < TQ) inputs, bf16: early queries have softmax support
    # too small to average fp8 quantization noise, so chunk 0 stays bf16
    # end-to-end. Chunk 1 (tokens TQ..T2) and later are fp8.
    # all activation/weight inputs are host-prearranged to the SBUF
    # [partition, o, t] layout so the DMAs read contiguous 8KB runs per
    # partition (the (o p) t -> p o t rearrange on the DRAM side decomposed
    # into 256B packets at ~16GB/s/engine and paced the startup)
    x0A_d = nc.dram_tensor("x0A", [128, CO, TQ], BF16, kind="ExternalInput")
    wqb_d = nc.dram_tensor("wqb", [128, CO, D], BF16, kind="ExternalInput")
    wkb_d = nc.dram_tensor("wkb", [128, CO, D], BF16, kind="ExternalInput")
    wvb_d = nc.dram_tensor("wvb", [128, CO, D], BF16, kind="ExternalInput")
    # chunk-1 input (fp8); x8T holds tokens T2..t_len, pair-major
    x08_d = nc.dram_tensor("x08", [128, CO, TQ], F8, kind="ExternalInput")
    wq8_d = nc.dram_tensor("wq8", [128, CO, D], F8, kind="ExternalInput")
    wk8_d = nc.dram_tensor("wk8", [128, CO, D], F8, kind="ExternalInput")
    wv8_d = nc.dram_tensor("wv8", [128, CO, D], F8, kind="ExternalInput")
    if n_pairs > 1:
        x8T_d = nc.dram_tensor("x8T", [128, (n_pairs - 1) * CO, T2], F8,
                               kind="ExternalInput")
    wp_d = nc.dram_tensor("wp", [D, C], BF16, kind="ExternalInput")
    bq_d = nc.dram_tensor("bq", [D, 1], F32, kind="ExternalInput")
    bv_d = nc.dram_tensor("bv", [D, 1], F32, kind="ExternalInput")
    # block layout: each proj unit writes one contiguous [128, 2*TQ] block
    # (2KB runs per partition vs 1KB strided); host reassembles [C, t]
    n_blk = (t_len // TQ) * (CO // 2)
    outP_d = nc.dram_tensor("outP", [n_blk * 128, 2 * TQ], BF16,
                            kind="ExternalOutput")

    with tile.TileContext(nc) as tc:
        with (
            tc.tile_pool(name="const", bufs=1) as cpool,
            tc.tile_pool(name="persist", bufs=1) as ppool,
            tc.tile_pool(name="work", bufs=2) as wpool,
            tc.tile_pool(name="ptiles", bufs=3) as pt_pool,
            tc.tile_pool(name="psum", bufs=1, space="PSUM") as psum,
        ):
            # ---- constants / weights -------------------------------------
            # wqb first so the very first (bf16 chunk-0 QKV) matmuls are
            # unblocked asap
            wqb_sb = cpool.tile([128, CO, D], BF16, name="wqb_sb")
            wkb_sb = cpool.tile([128, CO, D], BF16, name="wkb_sb")
            wvb_sb = cpool.tile([128, CO, D], BF16, name="wvb_sb")
            wq8_sb = cpool.tile([128, CO, D], F8, name="wq8_sb")
            wk8_sb = cpool.tile([128, CO, D], F8, name="wk8_sb")
            wv8_sb = cpool.tile([128, CO, D], F8, name="wv8_sb")
            wp_sb = cpool.tile([128, CO, D], BF16, name="wp_sb")
            nc.sync.dma_start(wqb_sb[:], wqb_d.ap())
            bq_sb = cpool.tile([D, 1], F32, name="bq_sb")
            bv_sb = cpool.tile([D, 1], F32, name="bv_sb")
            nc.sync.dma_start(bq_sb[:], bq_d.ap())
            nc.sync.dma_start(bv_sb[:], bv_d.ap())

            # denominator stationary tiles hold 16.0 so sums = 16*sum(p);
            # memset first so the PE warmup below is unblocked immediately
            sixt8 = cpool.tile([128, 2, 128], F8, name="sixt8")
            nc.vector.memset(sixt8[:], SW)
            sixtb = cpool.tile([128, 128], BF16, name="sixtb")
            nc.vector.memset(sixtb[:], SW)
            ident = cpool.tile([128, 128], BF16, name="ident")
            make_identity(nc, ident[:])
            # HAM/ifetch warmup: dummy DR matmuls while input DMAs land
            for wi in range(24):
                warm_ps = psum.tile([128, 128], F32,
                                    tag="s2A" if wi % 2 == 0 else "s2B",
                                    name="warm_ps", bufs=2)
                nc.tensor.matmul(warm_ps[:], sixt8[:], sixt8[:],
                                 start=True, stop=True, perf_mode=DR)

            # ---- persistent activations ----------------------------------
            kT8_sb = ppool.tile([128, t_len], F8, name="kT8_sb")
            kTb_sb = ppool.tile([128, T2], BF16, name="kTb_sb")
            v8_sb = ppool.tile([128, n_ttiles, D], F8, name="v8_sb")
            vb_sb = ppool.tile([128, 4, D], BF16, name="vb_sb")
            yT_sb = ppool.tile([128, t_len], BF16, name="yT_sb")


            x0A = wpool.tile([128, CO, TQ], BF16, tag="x0A", name="x0A", bufs=1)
            nc.sync.dma_start(x0A[:, 0:4, :], x0A_d.ap()[:, 0:4, :])
            nc.sync.dma_start(x0A[:, 4:CO, :], x0A_d.ap()[:, 4:CO, :])
            for w_sb, w_d in ((wkb_sb, wkb_d), (wvb_sb, wvb_d)):
                nc.sync.dma_start(w_sb[:], w_d.ap())
            xc0 = wpool.tile([128, CO, TQ], F8, tag="xc0", name="xc0", bufs=1)
            nc.sync.dma_start(xc0[:, 0:4, :], x08_d.ap()[:, 0:4, :])
            nc.sync.dma_start(xc0[:, 4:CO, :], x08_d.ap()[:, 4:CO, :])
            for w_sb, w_d in ((wq8_sb, wq8_d), (wk8_sb, wk8_d),
                              (wv8_sb, wv8_d)):
                nc.sync.dma_start(w_sb[:], w_d.ap())
            nc.sync.dma_start(
                wp_sb[:], wp_d.ap().rearrange("d (o j) -> d o j", j=128)
            )

            def make_proj_units(pj, half):
                # c_proj of pair pj, one chunk-half, as 4 deferred units.
                # Each unit computes TWO output column tiles into one 2-bank
                # PSUM tile (single pool allocation), one PSUM->SBUF copy
                # (bf16), one DMA. Units are drained one per inner-loop
                # iteration so the copies and DMAs spread out.
                units = []
                lo = pj * T2 + half * TQ
                for j0 in range(0, CO, 2):
                    def unit(lo=lo, j0=j0, half=half):
                        outc = wpool.tile([128, 2, TQ], BF16, tag="outc",
                                          name="outc", bufs=4)
                        for jj, ptag in ((0, "s2A"), (1, "s2B")):
                            oh = psum.tile([128, TQ], F32, tag=ptag,
                                           name="oh", bufs=2)
                            nc.tensor.matmul(
                                oh[:], wp_sb[:, j0 + jj, :],
                                yT_sb[:, lo : lo + TQ],
                                start=True, stop=True,
                            )
                            # split the unit's two copies across ACT and
                            # DVE so they run concurrently and free the
                            # scores' s2A/s2B PSUM slots sooner
                            if jj == 0:
                                nc.vector.tensor_copy(outc[:, jj, :], oh[:])
                            else:
                                nc.scalar.copy(outc[:, jj, :], oh[:])
                        ci = (lo // TQ) * (CO // 2) + j0 // 2
                        nc.sync.dma_start(
                            outP_d.ap()[ci * 128 : (ci + 1) * 128, :],
                            outc[:].rearrange("p a t -> p (a t)"),
                        )
                    units.append(unit)
                return units

            pair_state = {}

            def make_qkv_units(pj, xc, halves=(0, 1), xoff=0):
                # fp8 DoubleRow QKV for pair pj as single-bank deferred
                # units (kind x half); drained inside pair pj-1's loop.
                # xoff: token offset of xc's first column within the pair.
                t0p = pj * T2
                qT = wpool.tile([128, T2], BF16 if pj == 0 else F8,
                                tag="qT", name="qT", bufs=2)
                vT = wpool.tile([128, T2], BF16, tag="vT", name="vT", bufs=2)
                pair_state[pj] = (qT, vT)
                units = []
                # q first (next pair's scores need it at sp=0), then v
                # (transposes at sp=1), then k (scores si >= n_sA at sp~4)
                for kind in (0, 2, 1):   # 0=q, 1=k, 2=v
                    for half in halves:
                        def unit(kind=kind, half=half):
                            hs = slice(half * TQ, (half + 1) * TQ)
                            xs = slice(half * TQ - xoff,
                                       (half + 1) * TQ - xoff)
                            dst = psum.tile([128, TQ], F32,
                                            tag="s2A" if half == 0 else "s2B",
                                            name="qkvu", bufs=2)
                            w_sb = (wq8_sb, wk8_sb, wv8_sb)[kind]
                            for op in range(CO // 2):
                                o = 2 * op
                                nc.tensor.matmul(
                                    dst[:], w_sb[:, o : o + 2, :],
                                    xc[:, o : o + 2, xs],
                                    start=(op == 0), stop=(op == CO // 2 - 1),
                                    perf_mode=DR,
                                )
                            # unit epilogues ride ACT (bias via the
                            # activation bias port): the DVE queue stays
                            # short for the schrau exps, and the unit's
                            # s2A/s2B PSUM slot frees sooner
                            if kind == 0:
                                nc.scalar.activation(
                                    qT[:, hs], dst[:],
                                    mybir.ActivationFunctionType.Copy,
                                    bias=bq_sb[:, 0:1],
                                )
                            elif kind == 1:
                                nc.scalar.copy(
                                    kT8_sb[:, t0p + half * TQ
                                           : t0p + (half + 1) * TQ],
                                    dst[:],
                                )
                                if pj == 0:
                                    nc.vector.tensor_copy(
                                        kTb_sb[:, t0p + half * TQ
                                               : t0p + (half + 1) * TQ],
                                        dst[:],
                                    )
                            else:
                                nc.scalar.activation(
                                    vT[:, hs], dst[:],
                                    mybir.ActivationFunctionType.Copy,
                                    bias=bv_sb[:, 0:1],
                                )
                        units.append(unit)
                return units

            pending = []

            def drain_one():
                if pending:
                    pending.pop(0)()

            # pair 0's QKV inline: chunk 1's fp8 DR units first (xc0 is the
            # smaller, earliest-landing DMA), then chunk 0 in bf16
            qkv0_units = make_qkv_units(0, xc0, halves=(1,), xoff=TQ)
            qT0, vT0 = pair_state[0]
            A0 = slice(0, TQ)
            # fp8 B units first: xc0 (256KB fp8) lands well before the
            # 1MB bf16 x0A, so they fill the x0A DMA wait
            for u in qkv0_units:
                u()
            for kind, w_sb in ((0, wqb_sb), (1, wkb_sb), (2, wvb_sb)):
                dst0 = psum.tile([128, TQ], F32,
                                 tag="s2A" if kind != 1 else "s2B",
                                 name="dst0", bufs=2)
                for o in range(CO):
                    nc.tensor.matmul(
                        dst0[:], w_sb[:, o, :], x0A[:, o, :],
                        start=(o == 0), stop=(o == CO - 1),
                    )
                if kind == 0:
                    nc.vector.tensor_add(
                        qT0[:, A0], dst0[:],
                        bq_sb[:, 0:1].to_broadcast([D, TQ])
                    )
                elif kind == 1:
                    nc.vector.tensor_copy(kTb_sb[:, 0:TQ], dst0[:])
                    nc.vector.tensor_copy(kT8_sb[:, 0:TQ], dst0[:])
                else:
                    nc.vector.tensor_add(
                        vT0[:, A0], dst0[:],
                        bv_sb[:, 0:1].to_broadcast([D, TQ])
                    )

            for pc in range(n_pairs):
                t0 = pc * T2           # start of chunk A; chunk B at t0+TQ
                fp8av = FP8_AV
                # next pair's x chunk DMA + its deferred QKV units
                if pc + 1 < n_pairs:
                    t0n = (pc + 1) * T2
                    xc_next = wpool.tile([128, CO, T2], F8, tag="xc",
                                         name="xc", bufs=2)
                    nc.sync.dma_start(
                        xc_next[:], x8T_d.ap()[:, pc * CO : (pc + 1) * CO, :]
                    )
                    qkv_units_next = make_qkv_units(pc + 1, xc_next)
                else:
                    qkv_units_next = []

                qT_cur, vT_tmp = pair_state[pc]

                def emit_transposes():
                    for vg in range(2):
                        vt_ps = psum.tile([128, 4, 128], BF16,
                                          tag="s2A" if vg == 0 else "s2B",
                                          name="vt_ps", bufs=2)
                        for tt in range(4):
                            col = (vg * 4 + tt) * 128
                            nc.tensor.transpose(
                                vt_ps[:, tt, :], vT_tmp[:, col : col + 128],
                                ident[:],
                            )
                        base = pc * 8 + vg * 4
                        nc.vector.tensor_copy(
                            v8_sb[:, base : base + 4, :], vt_ps[:]
                        )
                        if pc == 0 and vg == 0:
                            # bf16 v for chunk 0's own (bf16) AV
                            nc.vector.tensor_copy(vb_sb[:], vt_ps[:])

                # ---- attention for the pair ------------------------------
                n_sA = (t0 + TQ) // 128        # s-tiles for chunk A
                n_sB = (t0 + T2) // 128        # s-tiles for chunk B
                yAB = psum.tile([128, T2], F32, tag="yAB", name="yAB", bufs=1)
                sumAB = psum.tile([128, T2], F32, tag="sumAB", name="sumAB",
                                  bufs=1)
                A, Bh = slice(0, TQ), slice(TQ, T2)
                recip = wpool.tile([128, T2], F32, tag="recip", name="recip",
                                   bufs=2)
                if pc == 0:
                    emit_transposes()   # pair 0's AV needs own v from si=0

                n_sp = n_sB // 2
                hyb = pc == 0   # chunk 0 (A half of pair 0) stays bf16

                def make_av(sp, p2, p2b=None):
                    # AV + denominator matmuls for step sp; emitted one
                    # iteration late (software pipelining) so the in-order PE
                    # queue never parks on an exp wait while later-emitted
                    # ready work exists.
                    si0 = 2 * sp
                    in_A = si0 < n_sA

                    def emit():
                        for hsl, n_s, last in (
                            (A, n_sA, in_A and sp == n_sA // 2 - 1),
                            (Bh, n_sB, sp == n_sp - 1),
                        ):
                            if hsl is A and not in_A:
                                continue
                            if hsl is A and hyb:
                                # chunk 0: bf16 p x bf16 v, no DoubleRow
                                for sl_i in range(2):
                                    si = si0 + sl_i
                                    nc.tensor.matmul(
                                        sumAB[:, A], sixtb[:], p2b[:, sl_i, :],
                                        start=(si == 0), stop=(si == n_sA - 1),
                                    )
                                    nc.tensor.matmul(
                                        yAB[:, A], vb_sb[:, si, :],
                                        p2b[:, sl_i, :],
                                        start=(si == 0), stop=(si == n_sA - 1),
                                    )
                                continue
                            nc.tensor.matmul(
                                sumAB[:, hsl], sixt8[:], p2[:, :, hsl],
                                start=(sp == 0), stop=last, perf_mode=DR,
                            )
                            nc.tensor.matmul(
                                yAB[:, hsl], v8_sb[:, si0 : si0 + 2, :],
                                p2[:, :, hsl],
                                start=(sp == 0), stop=last, perf_mode=DR,
                            )
                        if in_A and si0 == n_sA - 2:
                            # A-half AV complete: normalize early, then queue
                            # the A-half c_proj to fill later iterations
                            nc.vector.reciprocal_approx_fast(recip[:, A],
                                                             sumAB[:, A])
                            nc.vector.tensor_mul(
                                yT_sb[:, t0 : t0 + TQ], yAB[:, A], recip[:, A]
                            )
                            pending.extend(make_proj_units(pc, 0))

                    return emit

                av_prev = None
                av_prev2 = None
                for sp in range(n_sp):
                    if sp == max(0, n_sp - 7):
                        pending.extend(qkv_units_next)
                        qkv_units_next = []
                    si0 = 2 * sp
                    in_A = si0 < n_sA    # n_sA is a multiple of 4
                    p2 = pt_pool.tile([128, 2, T2], F8, tag="p28", name="p2",
                                      bufs=4)
                    p2b = None
                    if hyb and in_A:
                        p2b = pt_pool.tile([128, 2, TQ], BF16, tag="p2b",
                                           name="p2b", bufs=3)
                    for sl_i in range(2):
                        si = si0 + sl_i
                        s0 = si * 128
                        diagA = in_A and si >= n_sA - 4
                        diagB = si >= n_sB - 4
                        # diag tile j: cols < 128j are fully masked -- skip
                        # them in the score/exp stream (p2 prefix zeroed by
                        # the idle GpSimd) and mask only the 128-wide
                        # triangle
                        cA = 128 * (si - (n_sA - 4)) if diagA else 0
                        cB = 128 * (si - (n_sB - 4)) if diagB else 0
                        # scores A and B live in independent single-bank
                        # PSUM tiles with per-half exps, so each half's
                        # scores(si) only waits on its own exp(si-2)
                        def tri_zero(ap):
                            # zero p2's diagonal 128x128 triangle (q < k)
                            # post-exp on the idle GpSimd: replaces the PE
                            # mask matmul entirely. exp of unmasked scores
                            # can't overflow (bf16 range; fp8/u8 saturate).
                            nc.gpsimd.affine_select(
                                out=ap, in_=ap,
                                compare_op=mybir.AluOpType.is_ge, fill=0.0,
                                base=0, pattern=[[1, 128]],
                                channel_multiplier=-1,
                            )

                        if in_A:
                            sA = psum.tile([128, TQ], F32, tag="s2A",
                                           name="sA", bufs=2)
                            if cA:
                                nc.gpsimd.memset(
                                    (p2b if hyb else p2)[:, sl_i, 0:cA], 0.0
                                )
                            nc.tensor.matmul(sA[:, cA:],
                                             (kTb_sb if hyb else kT8_sb)
                                             [:, s0 : s0 + 128],
                                             qT_cur[:, cA:TQ], start=True,
                                             stop=True)
                            nc.scalar.activation(
                                (p2b[:, sl_i, cA:] if hyb
                                 else p2[:, sl_i, cA:TQ]),
                                sA[:, cA:],
                                mybir.ActivationFunctionType.Exp,
                                scale=exp_scale,
                            )
                            if diagA:
                                tri_zero((p2b if hyb else p2)
                                         [:, sl_i, cA : cA + 128])
                        sB = psum.tile([128, TQ], F32, tag="s2B",
                                       name="sB", bufs=2)
                        if cB:
                            nc.gpsimd.memset(p2[:, sl_i, TQ : TQ + cB], 0.0)
                        nc.tensor.matmul(sB[:, cB:],
                                         (kTb_sb if hyb else kT8_sb)
                                         [:, s0 : s0 + 128],
                                         qT_cur[:, TQ + cB : T2], start=True,
                                         stop=True)
                        if hyb:
                            # pair 0's B half (chunk 1) keeps the accurate
                            # ACT exp: it is the accuracy-thinnest fp8 chunk
                            nc.scalar.activation(
                                p2[:, sl_i, TQ + cB : T2], sB[:, cB:],
                                mybir.ActivationFunctionType.Exp,
                                scale=exp_scale,
                            )
                        else:
                            # fp8 pairs: Schraudolph exp on DVE. f32->u8
                            # conversion saturates ([0,255], negatives -> 0),
                            # so (s + B)*S1 needs no max op. Keeps the
                            # Scalar engine free for the A-half exps.
                            nc.vector.tensor_scalar(
                                p2[:, sl_i, TQ + cB : T2]
                                .bitcast(mybir.dt.uint8),
                                sB[:, cB:], SCH_BIAS / SCH_S1, SCH_S1,
                                op0=mybir.AluOpType.add,
                                op1=mybir.AluOpType.mult,
                            )
                        if diagB:
                            tri_zero(p2[:, sl_i, TQ + cB : TQ + cB + 128])
                    if av_prev2 is not None:
                        av_prev2()
                    av_prev2 = av_prev
                    av_prev = make_av(sp, p2, p2b)
                    drain_one()
                    if pc == 0 or (pc == n_pairs - 1 and sp >= n_sp - 4):
                        drain_one()   # ramp/tail regions are PE-light
                    if pc > 0 and sp == 1:
                        # own-pair v only needed from si >= n_sA; transposing
                        # here hides the vT copyback latency behind scores
                        emit_transposes()

                if av_prev2 is not None:
                    av_prev2()
                av_prev()
                # normalize B first so its DVE ops aren't queued behind the
                # drained units' copies, then flush the queue (QKV of the
                # next pair, proj leftovers)
                nc.vector.reciprocal_approx_fast(recip[:, Bh], sumAB[:, Bh])
                nc.vector.tensor_mul(
                    yT_sb[:, t0 + TQ : t0 + T2], yAB[:, Bh], recip[:, Bh]
                )
                while pending:
                    drain_one()
                pending.extend(make_proj_units(pc, 1))

            # last pair's remaining c_proj
            while pending:
                drain_one()

    nc.compile()
    return nc


def make_in_maps(x, w_attn, b_attn, w_proj, b_proj, t_len=T):
    """Shard + lay out the full inputs for the 8 cores."""
    x = np.asarray(x, dtype=np.float32).reshape(t_len, C)
    w_attn = np.asarray(w_attn, dtype=np.float32)
    b_attn = np.asarray(b_attn, dtype=np.float32)
    w_proj = np.asarray(w_proj, dtype=np.float32)

    bf = ml_dtypes.bfloat16
    f8 = ml_dtypes.float8_e4m3
    xT = np.ascontiguousarray(x.T)

    def _po(arr):
        # [C, X] -> SBUF layout [128, CO, X] (partition-major)
        return np.ascontiguousarray(
            arr.reshape(CO, 128, arr.shape[1]).transpose(1, 0, 2))

    T2 = 2 * TQ
    x0A = _po(xT[:, :TQ]).astype(bf)
    x08 = _po(xT[:, TQ:T2]).astype(f8)
    has8 = t_len > T2
    if has8:
        n_p1 = (t_len - T2) // T2
        x8T = np.concatenate(
            [_po(xT[:, T2 * (i + 1) : T2 * (i + 2)]) for i in range(n_p1)],
            axis=1).astype(f8)

    in_maps = []
    for h in range(N_CORES):
        sl = slice(h * D, (h + 1) * D)
        wq = np.ascontiguousarray((w_attn[sl, :] * SW).T)
        wk = np.ascontiguousarray((w_attn[C + h * D : C + (h + 1) * D, :] * SW).T)
        wv = np.ascontiguousarray((w_attn[2 * C + h * D : 2 * C + (h + 1) * D, :] * SW).T)
        wp = np.ascontiguousarray(w_proj[:, sl].T).astype(bf)
        m = {
            "x0A": x0A,
            "x08": x08,
            "wqb": _po(wq).astype(bf), "wkb": _po(wk).astype(bf),
            "wvb": _po(wv).astype(bf),
            "wp": wp,
            "bq": (b_attn[sl] * SW).reshape(D, 1).astype(np.float32),
            "bv": (b_attn[2 * C + h * D : 2 * C + (h + 1) * D] * SW)
                  .reshape(D, 1).astype(np.float32),
            "wq8": _po(wq).astype(f8),
            "wk8": _po(wk).astype(f8),
            "wv8": _po(wv).astype(f8),
        }
        if has8:
            m["x8T"] = x8T
        in_maps.append(m)
    return in_maps


_COMPILED = {}


def _get_compiled(t_len=T):
    if t_len not in _COMPILED:
        _COMPILED[t_len] = build(t_len)
    return _COMPILED[t_len]


def kernel(x, w_attn, b_attn, w_proj, b_proj, trace=False):
    nc = _get_compiled()
    in_maps = make_in_maps(x, w_attn, b_attn, w_proj, b_proj)
    res = bass_utils.run_bass_kernel_spmd(
        nc, in_maps, core_ids=list(range(N_CORES)), trace=trace
    )

    def _unblk(a):
        # [n_blk*128, 2*TQ] -> [C, T]: blocks are (chunk, jpair) major,
        # row c = (2*jpair + jj)*128 + p, col = chunk*TQ + t
        a = a.reshape(T // TQ, CO // 2, 128, 2, TQ)
        return a.transpose(1, 3, 2, 0, 4).reshape(C, T)

    acc = _unblk(res.results[0]["outP"].astype(np.float32))
    for h in range(1, N_CORES):
        acc += _unblk(res.results[h]["outP"].astype(np.float32))
    out = acc.T + np.asarray(b_proj, dtype=np.float32)
    out = np.ascontiguousarray(out, dtype=np.float32).reshape(B, T, C)
    if trace:
        kernel.last_exec_time_ns = res.exec_time_ns
        kernel.last_results = res
    return out

